# revision 1
# baseline (speedup 1.0000x reference)
"""Trainium2 Bass kernel for a 3-layer BiLSTM + ReLU + residual + LayerNorm.

Strategy (pure data parallel over 8 cores, 1024 batch rows per core):
  * "Transposed" layout on-chip: features on SBUF partitions, batch on the
    free dim.  Both directions fused on partitions (fwd = 0:64, bwd = 64:128)
    so every ScalarE/VectorE op runs with all 128 lanes busy.
  * Per timestep, per gate: one 128x128 block-diagonal recurrent matmul plus
    two 64-wide input-projection matmuls (col-tiled pairs) accumulate into a
    PSUM tile [g_fwd; g_bwd].  Sigmoid over a contiguous [i,f,o] PSUM span is
    a single ScalarE op; tanh(g), the c/h updates, and tanh(c) follow.
  * Layer outputs stream through DRAM as bf16 (the recurrence itself stays
    fp32); batch is processed as two interleaved chunks so the engines can
    overlap the sequential per-step dependency chain.
  * Final stage: PE transpose to natural layout + K=9 residual matmul into
    the same PSUM tile, LayerNorm stats via ScalarE accumulate outputs,
    normalize via per-partition tensor_scalar, DMA out natural-layout fp32.
"""

from contextlib import ExitStack

import numpy as np
import ml_dtypes

import concourse.bacc as bacc
import concourse.tile as tile
from concourse import mybir
from concourse.bass_utils import run_bass_kernel_spmd

F32 = mybir.dt.float32
BF16 = mybir.dt.bfloat16
AF = mybir.ActivationFunctionType
OP = mybir.AluOpType

NCORES = 8
BC = 1024               # batch rows per core
CHUNKS = 2
T = 64
H = 64
NL = 3
D2 = 2 * H              # 128
LN_EPS = 1e-5

# gate order in PyTorch weights: i, f, g, o  (rows g*H:(g+1)*H of w_ih/w_hh)
SIG_GATES = (0, 1, 3)   # i, f, o  -> sigmoid, held in one PSUM span
TANH_GATE = 2           # g        -> tanh


def _host_prep(x, w_ih, w_hh, b_ih, b_hh, w_res, b_res, ncores, bc):
    """Matmul-ready weight layouts (shared across cores) + per-core inputs."""
    x = np.asarray(x, np.float32)
    w_ih = np.asarray(w_ih, np.float32)
    w_hh = np.asarray(w_hh, np.float32)
    bias = np.asarray(b_ih, np.float32) + np.asarray(b_hh, np.float32)  # (NL,2,4H)
    w_res = np.asarray(w_res, np.float32)
    b_res = np.asarray(b_res, np.float32)
    t_len = x.shape[1]

    # Recurrent lhsT, K-major: rw[k, l, g, m] (block-diagonal over directions)
    rw = np.zeros((128, NL, 4, 128), np.float32)
    for l in range(NL):
        for g in range(4):
            gs = slice(g * H, (g + 1) * H)
            rw[0:64, l, g, 0:64] = w_hh[l, 0, gs, :].T
            rw[64:128, l, g, 64:128] = w_hh[l, 1, gs, :].T

    # Input-projection lhsT for layers 1,2 (bf16): pw[k, l-1, g, d, m]
    pw = np.zeros((128, NL - 1, 4, 2, 64), np.float32)
    for l in (1, 2):
        for g in range(4):
            gs = slice(g * H, (g + 1) * H)
            for d in range(2):
                pw[:, l - 1, g, d, :] = w_ih[l, d, gs, :].T
    pw = pw.astype(ml_dtypes.bfloat16)

    # Layer-0 projection lhsT with the bias folded into a ones-row (row 8)
    l0w = np.zeros((9, 4, 2, 64), np.float32)
    for g in range(4):
        gs = slice(g * H, (g + 1) * H)
        for d in range(2):
            l0w[0:8, g, d, :] = w_ih[0, d, gs, 0:8].T
            l0w[8, g, d, :] = bias[0, d, gs]

    # per-partition sigmoid-gate biases, layers 1,2 (fused dirs): br[p, idx]
    br = np.zeros((128, (NL - 1) * 3), np.float32)
    for l in (1, 2):
        for j, g in enumerate(SIG_GATES):
            gs = slice(g * H, (g + 1) * H)
            br[0:64, (l - 1) * 3 + j] = bias[l, 0, gs]
            br[64:128, (l - 1) * 3 + j] = bias[l, 1, gs]

    # g-gate bias per layer 1,2, per fused partition: gb[p, l-1]
    gb = np.zeros((128, NL - 1), np.float32)
    gs = slice(TANH_GATE * H, (TANH_GATE + 1) * H)
    for l in (1, 2):
        gb[0:64, l - 1] = bias[l, 0, gs]
        gb[64:128, l - 1] = bias[l, 1, gs]

    # residual rhs: wres[k, f] = w_res[f, k], row 8 = b_res
    wres = np.zeros((9, 128), np.float32)
    wres[0:8, :] = w_res.T
    wres[8, :] = b_res

    ident = np.eye(128, dtype=np.float32)

    # Per-core transposed-augmented input: xaug[k, t, b]
    xaug_cores = []
    for c in range(ncores):
        xc = x[c * bc:(c + 1) * bc]              # (bc, T, 8)
        xa = np.empty((9, t_len, bc), np.float32)
        xa[0:8] = xc.transpose(2, 1, 0)
        xa[8] = 1.0
        xaug_cores.append(xa)

    shared = dict(rw=rw, pw=pw, l0w=l0w, br=br, gb=gb, wres=wres, ident=ident)
    return shared, xaug_cores


def _emit(nc, tc, ctx, D, apply_gb, bc, t_len):
    bk = bc // CHUNKS
    fb = min(128, bk)         # final-stage block width (natural-layout rows)
    nb = bk // fb             # blocks per chunk per timestep
    strip = min(8, t_len)

    sbC = ctx.enter_context(tc.tile_pool(name="consts", bufs=1))
    sbA = ctx.enter_context(tc.tile_pool(name="workA", bufs=3))
    sbB = ctx.enter_context(tc.tile_pool(name="workB", bufs=2))
    sbS = ctx.enter_context(tc.tile_pool(name="state", bufs=1))
    sbZ = ctx.enter_context(tc.tile_pool(name="zhold", bufs=strip + 2))
    ps = ctx.enter_context(tc.tile_pool(name="ps", bufs=1, space="PSUM"))

    def const_tile(shape, dtype, key):
        t = sbC.tile(shape, dtype, name=f"c_{key}", tag=f"c_{key}")
        nc.sync.dma_start(out=t, in_=D[key])
        return t

    rw_sb = const_tile([128, NL, 4, 128], F32, "rw")
    pw_sb = const_tile([128, NL - 1, 4, 2, 64], BF16, "pw")
    l0w_sb = const_tile([9, 4, 2, 64], F32, "l0w")
    br_sb = const_tile([128, (NL - 1) * 3], F32, "br")
    gb_sb = const_tile([128, NL - 1], F32, "gb")
    wres_sb = const_tile([9, 128], F32, "wres")
    ident_sb = const_tile([128, 128], F32, "ident")
    gamma_sb = beta_sb = None
    if apply_gb:
        gamma_sb = const_tile([fb, 128], F32, "gammab")
        beta_sb = const_tile([fb, 128], F32, "betab")
    ones_sb = sbC.tile([1, bk], F32)
    nc.vector.memset(ones_sb, 1.0)
    eps_sb = sbC.tile([128, 1], F32)
    nc.vector.memset(eps_sb, LN_EPS)

    O = [D[f"o{i}"] for i in range(NL)]
    xaug = D["xaug"]
    out_d = D["out"]

    h_prev = [None] * CHUNKS
    c_st = [None] * CHUNKS

    def issue_inp(cc, l, k):
        # issued ahead of the consuming step so input reads enter the DMA
        # queue before the chain-tail output writes (no head-of-line block)
        c0 = cc * bk
        cols = slice(c0, c0 + bk)
        rt = t_len - 1 - k
        if l == 0:
            inp_f = sbA.tile([9, bk], F32, tag=f"inf{cc}", bufs=4, name="inp_f")
            nc.sync.dma_start(out=inp_f, in_=xaug[:, k, cols])
            inp_b = sbA.tile([9, bk], F32, tag=f"inb{cc}", bufs=4, name="inp_b")
            nc.sync.dma_start(out=inp_b, in_=xaug[:, rt, cols])
        else:
            inp_f = sbA.tile([128, bk], BF16, tag=f"inf{cc}", bufs=4, name="inp_f")
            nc.sync.dma_start(out=inp_f, in_=O[l - 1][:, k, cols])
            inp_b = sbA.tile([128, bk], BF16, tag=f"inb{cc}", bufs=4, name="inp_b")
            nc.sync.dma_start(out=inp_b, in_=O[l - 1][:, rt, cols])
        return inp_f, inp_b

    def lstm_step(cc, l, k, inp_f, inp_b):
        c0 = cc * bk
        cols = slice(c0, c0 + bk)
        rt = t_len - 1 - k

        P_ifo = ps.tile([128, 3, bk], F32, tag=f"pifo{cc}")
        P_g = ps.tile([128, bk], F32, tag=f"pg{cc}")

        def gate_mms(out_ap, g, j):
            calls = []  # (out, lhsT, rhs, tile_position, partition_range)
            w = l0w_sb if l == 0 else pw_sb
            wf = w[:, g, 0, :] if l == 0 else w[:, l - 1, g, 0, :]
            wb = w[:, g, 1, :] if l == 0 else w[:, l - 1, g, 1, :]
            calls.append((out_ap[0:64, :], wf, inp_f, (0, 0), (0, 64)))
            calls.append((out_ap[64:128, :], wb, inp_b, (0, 64), (64, 128)))
            if k > 0:
                calls.append((out_ap, rw_sb[:, l, g, :], h_prev[cc], None,
                              (0, 128)))
            n = len(calls)
            for i, (o, lh, rh, tp, rng) in enumerate(calls):
                # start: this call's partitions not all covered by earlier calls
                covered = set()
                for _, _, _, _, r in calls[:i]:
                    covered.update(range(*r))
                start = not set(range(*rng)).issubset(covered)
                # stop: no later call touches this call's partitions
                stop = not any(max(rng[0], r[0]) < min(rng[1], r[1])
                               for _, _, _, _, r in calls[i + 1:])
                # skip_group_check: the executing-sim group checker
                # mis-addresses partition-based PSUM offsets (tensor rows
                # != 16KB); data semantics are still simulated exactly.
                nc.tensor.matmul(o, lh, rh, start=start, stop=stop,
                                 tile_position=tp, skip_group_check=True)

        for j, g in enumerate(SIG_GATES):
            gate_mms(P_ifo[:, j, :], g, j)
        gate_mms(P_g, TANH_GATE, None)

        S_ifo = sbB.tile([128, 3, bk], F32, tag=f"sifo{cc}", bufs=3)
        S_g = sbB.tile([128, bk], F32, tag=f"sg{cc}")

        def sig(j):
            if l > 0:
                idx = (l - 1) * 3 + j
                nc.scalar.activation(out=S_ifo[:, j, :], in_=P_ifo[:, j, :],
                                     func=AF.Sigmoid,
                                     bias=br_sb[:, idx:idx + 1])
            else:
                nc.scalar.activation(out=S_ifo[:, j, :], in_=P_ifo[:, j, :],
                                     func=AF.Sigmoid)

        sig(0)                                                    # i
        if l > 0:
            nc.scalar.activation(out=S_g, in_=P_g, func=AF.Tanh,
                                 bias=gb_sb[:, l - 1:l])
        else:
            nc.scalar.activation(out=S_g, in_=P_g, func=AF.Tanh)
        sig(1)                                                    # f
        sig(2)                                                    # o

        if k == 0:
            c = sbS.tile([128, bk], F32, tag=f"c{cc}")
            c_st[cc] = c
            nc.vector.tensor_mul(c, S_ifo[:, 0, :], S_g)          # c = i*g
        else:
            c = c_st[cc]
            tmp = sbB.tile([128, bk], F32, tag=f"tmp{cc}")
            nc.gpsimd.tensor_mul(tmp, S_ifo[:, 0, :], S_g)        # i*g (POOL)
            nc.vector.tensor_mul(c, S_ifo[:, 1, :], c)            # f*c
            nc.vector.tensor_add(c, c, tmp)
        return S_ifo, c

    def lstm_step_ph2(cc, l, k, S_ifo, c):
        # second phase emitted after the other chunk's phase 1 so the
        # ScalarE FIFO never head-of-line blocks on tanh(c) while the other
        # chunk's (ready) sigmoid sits behind it
        c0 = cc * bk
        cols = slice(c0, c0 + bk)
        rt = t_len - 1 - k
        Tc = sbB.tile([128, bk], F32, tag=f"tc{cc}")
        nc.scalar.activation(out=Tc, in_=c, func=AF.Tanh)
        h = sbA.tile([128, bk], F32, tag=f"h{cc}")
        nc.vector.tensor_mul(h, S_ifo[:, 2, :], Tc)               # h = o*tanh(c)
        h_prev[cc] = h

        # cast + store time-ordered halves: fwd half at t=k, bwd half at t=rt
        h_bf = sbA.tile([128, bk], BF16, tag=f"hbf{cc}")
        nc.gpsimd.tensor_copy(out=h_bf, in_=h)
        nc.sync.dma_start(out=O[l][0:64, k, cols], in_=h_bf[0:64, :])
        nc.sync.dma_start(out=O[l][64:128, rt, cols], in_=h_bf[64:128, :])

    PF = min(2, t_len - 1)
    for l in range(NL):
        pend = {}
        for kk in range(PF):
            for cc in range(CHUNKS):
                pend[(cc, kk)] = issue_inp(cc, l, kk)
        for k in range(t_len):
            ph1 = {}
            for cc in range(CHUNKS):
                if k + PF < t_len:
                    pend[(cc, k + PF)] = issue_inp(cc, l, k + PF)
                inp_f, inp_b = pend.pop((cc, k))
                ph1[cc] = lstm_step(cc, l, k, inp_f, inp_b)
            for cc in range(CHUNKS):
                S_ifo, c = ph1[cc]
                lstm_step_ph2(cc, l, k, S_ifo, c)

    # ---- final stage: relu + residual + LayerNorm + transpose to natural ----
    sums = [sbS.tile([fb, nb, t_len], F32, tag=f"sums{cc}", name=f"sums{cc}")
            for cc in range(CHUNKS)]
    sumsq = [sbS.tile([fb, nb, t_len], F32, tag=f"sumsq{cc}", name=f"sumsq{cc}")
             for cc in range(CHUNKS)]

    def issue_fin(cc, t):
        c0 = cc * bk
        cols = slice(c0, c0 + bk)
        o2t = sbA.tile([128, bk], BF16, tag=f"inf{cc}", bufs=4, name="o2t")
        nc.sync.dma_start(out=o2t, in_=O[NL - 1][:, t, cols])
        xt = sbA.tile([9, bk], F32, tag=f"inb{cc}", bufs=4, name="xt")
        nc.sync.dma_start(out=xt, in_=xaug[:, t, cols])
        return o2t, xt

    def final_t(cc, t, zs, o2t, xt):
        c0 = cc * bk
        cols = slice(c0, c0 + bk)
        relu4 = sbB.tile([128, bk], F32, tag=f"relu{cc}")
        nc.gpsimd.tensor_scalar_max(relu4, o2t, 0.0)
        # one accumulation group for the whole bank: transpose overwrites its
        # quarter (pending-zero from the single start), residual accumulates
        psZ = ps.tile([fb, nb, 128], F32, tag=f"pg{cc}")
        for bi in range(nb):
            bs = slice(bi * fb, (bi + 1) * fb)
            nc.tensor.matmul(psZ[:, bi, :], relu4[:, bs], ident_sb,
                             is_transpose=True, start=(bi == 0), stop=False,
                             skip_group_check=True)
            nc.tensor.matmul(psZ[:, bi, :], xt[:, bs], wres_sb,
                             start=False, stop=(bi == nb - 1),
                             skip_group_check=True)
        z = sbZ.tile([fb, nb, 128], F32, tag=f"z{cc}")
        z2 = sbB.tile([fb, nb, 128], F32, tag=f"z2{cc}")
        for bi in range(nb):
            nc.scalar.activation(out=z[:, bi, :], in_=psZ[:, bi, :],
                                 func=AF.Identity,
                                 accum_out=sums[cc][:, bi, t:t + 1])
            nc.scalar.activation(out=z2[:, bi, :], in_=psZ[:, bi, :],
                                 func=AF.Square,
                                 accum_out=sumsq[cc][:, bi, t:t + 1])
        zs.append((t, z))

    def final_strip_norm(cc, t0, zs):
        c0 = cc * bk
        ss = slice(t0, t0 + strip)
        mu = sbB.tile([fb, nb, strip], F32, tag=f"mu{cc}")
        nc.vector.tensor_scalar_mul(mu, sums[cc][:, :, ss], 1.0 / D2)
        var = sbB.tile([fb, nb, strip], F32, tag=f"var{cc}")
        nc.vector.tensor_scalar_mul(var, sumsq[cc][:, :, ss], 1.0 / D2)
        mu2 = sbB.tile([fb, nb, strip], F32, tag=f"mu2{cc}")
        nc.vector.tensor_mul(mu2, mu, mu)
        nc.vector.tensor_sub(var, var, mu2)
        sd = sbB.tile([fb, nb, strip], F32, tag=f"sd{cc}")
        nc.scalar.activation(out=sd, in_=var, func=AF.Sqrt,
                             bias=eps_sb[0:fb, 0:1])
        rstd = sbB.tile([fb, nb, strip], F32, tag=f"rstd{cc}")
        nc.vector.reciprocal(rstd, sd)
        nmr = sbB.tile([fb, nb, strip], F32, tag=f"nmr{cc}")
        nc.vector.scalar_tensor_tensor(nmr, mu, -1.0, rstd,
                                       op0=OP.mult, op1=OP.mult)
        for (t, z) in zs:
            ti = t - t0
            for bi in range(nb):
                on = sbA.tile([fb, 128], F32, tag=f"on{cc}")
                nc.vector.tensor_scalar(on, z[:, bi, :],
                                        rstd[:, bi, ti:ti + 1],
                                        nmr[:, bi, ti:ti + 1],
                                        op0=OP.mult, op1=OP.add)
                if apply_gb:
                    nc.vector.tensor_mul(on, on, gamma_sb)
                    nc.vector.tensor_add(on, on, beta_sb)
                b0 = c0 + bi * fb
                nc.sync.dma_start(out=out_d[b0:b0 + fb, t, :], in_=on)

    fpend = {}
    for tt in range(PF):
        for cc in range(CHUNKS):
            fpend[(cc, tt)] = issue_fin(cc, tt)
    for t0 in range(0, t_len, strip):
        zstrip = [[] for _ in range(CHUNKS)]
        for t in range(t0, t0 + strip):
            for cc in range(CHUNKS):
                if t + PF < t_len:
                    fpend[(cc, t + PF)] = issue_fin(cc, t + PF)
                o2t, xt = fpend.pop((cc, t))
                final_t(cc, t, zstrip[cc], o2t, xt)
        for cc in range(CHUNKS):
            final_strip_norm(cc, t0, zstrip[cc])


def build(apply_gb=False, bc=BC, t_len=T, num_devices=NCORES):
    nc = bacc.Bacc("TRN2", target_bir_lowering=False, debug=False,
                   num_devices=num_devices)
    fb = min(128, bc // CHUNKS)
    D = {}

    def inp(name, shape, dtype=F32):
        D[name] = nc.dram_tensor(name, shape, dtype, kind="ExternalInput").ap()

    inp("xaug", [9, t_len, bc])
    inp("rw", [128, NL, 4, 128])
    inp("pw", [128, NL - 1, 4, 2, 64], BF16)
    inp("l0w", [9, 4, 2, 64])
    inp("br", [128, (NL - 1) * 3])
    inp("gb", [128, NL - 1])
    inp("wres", [9, 128])
    inp("ident", [128, 128])
    if apply_gb:
        inp("gammab", [fb, 128])
        inp("betab", [fb, 128])
    for i in range(NL):
        D[f"o{i}"] = nc.dram_tensor(f"o{i}", [128, t_len, bc], BF16).ap()
    D["out"] = nc.dram_tensor("out", [bc, t_len, 128], F32,
                              kind="ExternalOutput").ap()

    with tile.TileContext(nc) as tc:
        with ExitStack() as ctx:
            _emit(nc, tc, ctx, D, apply_gb, bc, t_len)
    nc.compile()
    return nc


_BUILD_CACHE = {}


def kernel(x, w_ih, w_hh, b_ih, b_hh, w_res, b_res, ln_gamma, ln_beta):
    ln_gamma = np.asarray(ln_gamma, np.float32)
    ln_beta = np.asarray(ln_beta, np.float32)
    apply_gb = not (np.all(ln_gamma == 1.0) and np.all(ln_beta == 0.0))

    shared, xaug_cores = _host_prep(x, w_ih, w_hh, b_ih, b_hh, w_res, b_res,
                                    NCORES, BC)
    if apply_gb not in _BUILD_CACHE:
        _BUILD_CACHE[apply_gb] = build(apply_gb)
    nc = _BUILD_CACHE[apply_gb]

    in_maps = []
    for c in range(NCORES):
        m = dict(shared)
        m["xaug"] = xaug_cores[c]
        if apply_gb:
            fb = min(128, BC // CHUNKS)
            m["gammab"] = np.ascontiguousarray(
                np.broadcast_to(ln_gamma, (fb, 128)).astype(np.float32))
            m["betab"] = np.ascontiguousarray(
                np.broadcast_to(ln_beta, (fb, 128)).astype(np.float32))
        in_maps.append(m)

    res = run_bass_kernel_spmd(nc, in_maps, core_ids=list(range(NCORES)))
    out = np.concatenate([res.results[c]["out"] for c in range(NCORES)], axis=0)
    return np.ascontiguousarray(out.astype(np.float32))



# revision 5
# speedup vs baseline: 2.1064x; 2.1064x over previous
"""Trainium2 Bass kernel for a 3-layer BiLSTM + ReLU + residual + LayerNorm.

Strategy (pure data parallel over 8 cores, 1024 batch rows per core):
  * Transposed on-chip layout: features on SBUF partitions, batch on the
    free dim; both directions fused on partitions (fwd 0:64, bwd 64:128).
  * All matmul operands are bf16 (1 PE cycle/row vs 4 for fp32); PSUM and
    the cell state c stay fp32.  h is produced directly in bf16 by the DVE
    (2x mode) and doubles as the DRAM layer output.
  * Layer outputs stream through DRAM in PRODUCTION order (bwd half stored
    at its production step, not its time index); the next layer's strip
    loads undo the reversal with negative-stride DMA reads.
  * All DMA is strip-granular (S=8 timesteps per transfer) to amortize the
    per-issue DGE cost; batch is processed as two interleaved 512-column
    chunks so engines overlap across the sequential per-step chain.
  * Layer 0 folds fwd+bwd input projections and both biases into a single
    K=17 matmul per gate, so its i/f/o sigmoid is a single ScalarE op.
  * Final stage: residual matmul in transposed layout, fused
    relu+add (scalar_tensor_tensor), bf16 PE transpose to natural layout,
    LN stats via DVE accumulate/reduce ops, normalize on Pool.
"""

from contextlib import ExitStack

import numpy as np
import ml_dtypes

import concourse.bacc as bacc
import concourse.tile as tile
from concourse import mybir
from concourse.bass_utils import run_bass_kernel_spmd

F32 = mybir.dt.float32
BF16 = mybir.dt.bfloat16
AF = mybir.ActivationFunctionType
OP = mybir.AluOpType

NCORES = 8
BC = 1024               # batch rows per core
CHUNKS = 2
T = 64
H = 64
NL = 3
D2 = 2 * H              # 128
S = 8                   # timesteps per DMA strip
LN_EPS = 1e-5

# gate order in PyTorch weights: i, f, g, o  (rows g*H:(g+1)*H of w_ih/w_hh)
SIG_GATES = (0, 1, 3)   # i, f, o  -> sigmoid, held in one PSUM span
TANH_GATE = 2           # g        -> tanh


def _host_prep(x, w_ih, w_hh, b_ih, b_hh, w_res, b_res, ncores, bc):
    """Matmul-ready bf16 weight layouts (shared) + per-core inputs."""
    x = np.asarray(x, np.float32)
    w_ih = np.asarray(w_ih, np.float32)
    w_hh = np.asarray(w_hh, np.float32)
    bias = np.asarray(b_ih, np.float32) + np.asarray(b_hh, np.float32)  # (NL,2,4H)
    w_res = np.asarray(w_res, np.float32)
    b_res = np.asarray(b_res, np.float32)
    t_len = x.shape[1]
    bf = ml_dtypes.bfloat16

    # Recurrent lhsT, K-major, block-diagonal over directions
    rw = np.zeros((128, NL, 4, 128), np.float32)
    for l in range(NL):
        for g in range(4):
            gs = slice(g * H, (g + 1) * H)
            rw[0:64, l, g, 0:64] = w_hh[l, 0, gs, :].T
            rw[64:128, l, g, 64:128] = w_hh[l, 1, gs, :].T

    # Input-projection lhsT for layers 1,2: pw[k, l-1, g, d, m]
    pw = np.zeros((128, NL - 1, 4, 2, 64), np.float32)
    for l in (1, 2):
        for g in range(4):
            gs = slice(g * H, (g + 1) * H)
            for d in range(2):
                pw[:, l - 1, g, d, :] = w_ih[l, d, gs, :].T

    # Layer-0 stacked projection lhsT: rows 0:8 fwd x feats -> cols 0:64,
    # row 8 ones -> both biases, rows 9:17 bwd x feats -> cols 64:128
    l0w = np.zeros((17, 4, 128), np.float32)
    for g in range(4):
        gs = slice(g * H, (g + 1) * H)
        l0w[0:8, g, 0:64] = w_ih[0, 0, gs, 0:8].T
        l0w[8, g, 0:64] = bias[0, 0, gs]
        l0w[8, g, 64:128] = bias[0, 1, gs]
        l0w[9:17, g, 64:128] = w_ih[0, 1, gs, 0:8].T

    # per-partition sigmoid-gate act biases, layers 1,2: br[p, l-1, j]
    br = np.zeros((128, NL - 1, 3), np.float32)
    for l in (1, 2):
        for j, g in enumerate(SIG_GATES):
            gs = slice(g * H, (g + 1) * H)
            br[0:64, l - 1, j] = bias[l, 0, gs]
            br[64:128, l - 1, j] = bias[l, 1, gs]

    # g-gate act bias per layer 1,2: gb[p, l-1]
    gb = np.zeros((128, NL - 1), np.float32)
    gs = slice(TANH_GATE * H, (TANH_GATE + 1) * H)
    for l in (1, 2):
        gb[0:64, l - 1] = bias[l, 0, gs]
        gb[64:128, l - 1] = bias[l, 1, gs]

    # residual lhsT (transposed layout): rows 0:8 = w_res.T, row 8 = b_res
    wres = np.zeros((9, 128), np.float32)
    wres[0:8, :] = w_res.T
    wres[8, :] = b_res

    ident = np.eye(128, dtype=np.float32)

    # Per-core transposed-augmented input xs[k, t, b] (bf16):
    # rows 0:8 x(t) feats, row 8 ones, rows 9:17 x(T-1-t) feats
    xs_cores = []
    for c in range(ncores):
        xc = x[c * bc:(c + 1) * bc]              # (bc, T, 8)
        xa = np.empty((17, t_len, bc), np.float32)
        xa[0:8] = xc.transpose(2, 1, 0)
        xa[8] = 1.0
        xa[9:17] = xc[:, ::-1].transpose(2, 1, 0)
        xs_cores.append(xa.astype(bf))

    shared = dict(rw=rw.astype(bf), pw=pw.astype(bf), l0w=l0w.astype(bf),
                  br=br, gb=gb, wres=wres.astype(bf), ident=ident)
    return shared, xs_cores


def _mm_group(nc, calls):
    """Emit a PSUM accumulation group with per-call start/stop coverage."""
    for i, (o, lh, rh, tp, rng) in enumerate(calls):
        covered = set()
        for _, _, _, _, r in calls[:i]:
            covered.update(range(*r))
        start = not set(range(*rng)).issubset(covered)
        stop = not any(max(rng[0], r[0]) < min(rng[1], r[1])
                       for _, _, _, _, r in calls[i + 1:])
        # skip_group_check: the executing-sim group checker mis-addresses
        # partition-based PSUM offsets; data semantics are still exact.
        nc.tensor.matmul(o, lh, rh, start=start, stop=stop,
                         tile_position=tp, skip_group_check=True)


def _emit(nc, tc, ctx, D, apply_gb, bc, t_len):
    bk = bc // CHUNKS
    fb = min(128, bk)         # final-stage block width (natural-layout rows)
    nb = bk // fb             # blocks per chunk per timestep
    nstrip = t_len // S

    sbC = ctx.enter_context(tc.tile_pool(name="consts", bufs=1))
    sbI = ctx.enter_context(tc.tile_pool(name="inp", bufs=2))
    sbH = ctx.enter_context(tc.tile_pool(name="hbuf", bufs=2))
    sbB = ctx.enter_context(tc.tile_pool(name="work", bufs=2))
    sbS = ctx.enter_context(tc.tile_pool(name="state", bufs=1))
    sbZ = ctx.enter_context(tc.tile_pool(name="zhold", bufs=S + 1))
    ps = ctx.enter_context(tc.tile_pool(name="ps", bufs=1, space="PSUM"))

    def const_tile(shape, dtype, key):
        t = sbC.tile(shape, dtype, name=f"c_{key}", tag=f"c_{key}")
        nc.sync.dma_start(out=t, in_=D[key])
        return t

    rw_sb = const_tile([128, NL, 4, 128], BF16, "rw")
    pw_sb = const_tile([128, NL - 1, 4, 2, 64], BF16, "pw")
    l0w_sb = const_tile([17, 4, 128], BF16, "l0w")
    br_sb = const_tile([128, NL - 1, 3], F32, "br")
    gb_sb = const_tile([128, NL - 1], F32, "gb")
    wres_sb = const_tile([9, 128], BF16, "wres")
    ident_sb = const_tile([128, 128], F32, "ident")
    gamma_sb = beta_sb = None
    if apply_gb:
        gamma_sb = const_tile([fb, 128], F32, "gammab")
        beta_sb = const_tile([fb, 128], F32, "betab")
    eps_sb = sbC.tile([128, 1], F32, name="eps")
    nc.vector.memset(eps_sb, LN_EPS)

    P = [D[f"o{i}"] for i in range(NL)]
    xs = D["xs"]
    out_d = D["out"]

    h_prev = [None] * CHUNKS   # AP of last step's h (slice of a strip buffer)
    c_st = [None] * CHUNKS

    def rev(t_hi):
        """slice reading DRAM time t_hi, t_hi-1, ..., t_hi-S+1."""
        lo = t_hi - S
        return slice(t_hi, None if lo < 0 else lo, -1)

    def issue_loads(l, cc, si):
        """Issue strip-granular input loads for strip si of layer l."""
        k0 = si * S
        cols = slice(cc * bk, cc * bk + bk)
        if l == 0:
            xt = sbI.tile([17, S, bk], BF16, tag=f"x{cc}", name="xt")
            nc.sync.dma_start(out=xt, in_=xs[:, k0:k0 + S, cols])
            return (xt,)
        src = P[l - 1]
        F = sbI.tile([128, S, bk], BF16, tag=f"F{cc}", name="Ft")
        nc.sync.dma_start(out=F[0:64], in_=src[0:64, k0:k0 + S, cols])
        nc.sync.dma_start(out=F[64:128],
                          in_=src[64:128, rev(t_len - 1 - k0), cols])
        B = sbI.tile([128, S, bk], BF16, tag=f"B{cc}", name="Bt")
        nc.sync.dma_start(out=B[0:64],
                          in_=src[0:64, rev(t_len - 1 - k0), cols])
        nc.sync.dma_start(out=B[64:128], in_=src[64:128, k0:k0 + S, cols])
        return (F, B)

    def lstm_ph1(cc, l, k, j, tiles):
        # per-gate PSUM tiles (1 bank each): gate i's next-step matmuls only
        # wait for gate i's own sigmoid to drain, not the whole step's acts
        P4 = [ps.tile([128, bk], F32, tag=f"p{gi}_{cc}", name=f"P{gi}")
              for gi in range(4)]

        def gate_calls(out_ap, g):
            if l == 0:
                (xt,) = tiles
                calls = [(out_ap, l0w_sb[:, g, :], xt[:, j, :], None, (0, 128))]
            else:
                F, B = tiles
                calls = [
                    (out_ap[0:64, :], pw_sb[:, l - 1, g, 0, :], F[:, j, :],
                     (0, 0), (0, 64)),
                    (out_ap[64:128, :], pw_sb[:, l - 1, g, 1, :], B[:, j, :],
                     (0, 64), (64, 128)),
                ]
            if k > 0:
                calls.append((out_ap, rw_sb[:, l, g, :], h_prev[cc], None,
                              (0, 128)))
            _mm_group(nc, calls)

        for jg, g in enumerate(SIG_GATES):
            gate_calls(P4[jg], g)
        gate_calls(P4[3], TANH_GATE)

        S4 = [sbB.tile([128, bk], BF16, tag=f"s{gi}_{cc}", name=f"S{gi}")
              for gi in range(4)]
        for jg in range(3):
            bias = 0.0 if l == 0 else br_sb[:, l - 1, jg:jg + 1]
            nc.scalar.activation(out=S4[jg], in_=P4[jg], func=AF.Sigmoid,
                                 bias=bias)
        nc.scalar.activation(out=S4[3], in_=P4[3], func=AF.Tanh,
                             bias=0.0 if l == 0 else gb_sb[:, l - 1:l])

        # chunk 0's c-chain runs on DVE, chunk 1's on Pool, so the two
        # chunks' serial c-updates execute on different engines in parallel
        eng = nc.vector if cc == 0 else nc.gpsimd
        if k == 0:
            c = sbS.tile([128, bk], F32, tag=f"c{cc}", name="c")
            c_st[cc] = c
            eng.tensor_mul(c, S4[0], S4[3])                       # c = i*g
        else:
            c = c_st[cc]
            tmp = sbB.tile([128, bk], BF16, tag=f"tmp{cc}", name="tmp")
            eng.tensor_mul(tmp, S4[0], S4[3])                     # i*g
            eng.tensor_mul(c, S4[1], c)                           # f*c
            eng.tensor_add(c, c, tmp)
        return S4, c

    def lstm_ph2(cc, S4, c, j, hb):
        # emitted after the other chunk's phase 1 so the ScalarE FIFO never
        # head-of-line blocks on tanh(c) while a ready sigmoid sits behind it
        Tc = sbB.tile([128, bk], BF16, tag=f"tc{cc}", name="Tc")
        nc.scalar.activation(out=Tc, in_=c, func=AF.Tanh)
        hs = hb[:, j, :]
        nc.vector.tensor_mul(hs, S4[2], Tc)            # h = o*tanh(c), bf16 2x
        h_prev[cc] = hs

    # ---------------- LSTM layers ----------------
    for l in range(NL):
        pend = {cc: issue_loads(l, cc, 0) for cc in range(CHUNKS)}
        for si in range(nstrip):
            k0 = si * S
            nxt = {cc: issue_loads(l, cc, si + 1) for cc in range(CHUNKS)} \
                if si + 1 < nstrip else None
            hb = {}
            for cc in range(CHUNKS):
                hb[cc] = sbH.tile([128, S, bk], BF16, tag=f"hb{cc}",
                                  name="hb")
            for j in range(S):
                k = k0 + j
                ph1_res = {}
                for cc in range(CHUNKS):
                    ph1_res[cc] = lstm_ph1(cc, l, k, j, pend[cc])
                for cc in range(CHUNKS):
                    S4, c = ph1_res[cc]
                    lstm_ph2(cc, S4, c, j, hb[cc])
            for cc in range(CHUNKS):
                cols = slice(cc * bk, cc * bk + bk)
                nc.sync.dma_start(out=P[l][:, k0:k0 + S, cols], in_=hb[cc])
            if nxt is not None:
                pend = nxt

    # ---- final stage: relu + residual + LayerNorm + transpose to natural ----
    def fin_loads(cc, si):
        t0 = si * S
        cols = slice(cc * bk, cc * bk + bk)
        o2 = sbI.tile([128, S, bk], BF16, tag=f"F{cc}", name="o2")
        nc.sync.dma_start(out=o2[0:64], in_=P[2][0:64, t0:t0 + S, cols])
        nc.sync.dma_start(out=o2[64:128],
                          in_=P[2][64:128, rev(t_len - 1 - t0), cols])
        xr = sbI.tile([17, S, bk], BF16, tag=f"x{cc}", name="xr")
        nc.sync.dma_start(out=xr[0:9], in_=xs[0:9, t0:t0 + S, cols])
        return (o2, xr)

    fpend = {cc: fin_loads(cc, 0) for cc in range(CHUNKS)}
    Sh = S // 2    # half-strip: A/B output buffers so stores pipeline

    for si in range(nstrip):
        t0 = si * S
        fnxt = {cc: fin_loads(cc, si + 1) for cc in range(CHUNKS)} \
            if si + 1 < nstrip else None
        obufs = {}
        for cc in range(CHUNKS):
            obufs[cc] = (
                sbZ.tile([fb, nb, Sh, 128], BF16, tag=f"obA{cc}", bufs=1,
                         name="obufA"),
                sbZ.tile([fb, nb, Sh, 128], BF16, tag=f"obB{cc}", bufs=1,
                         name="obufB"),
            )
        for j in range(S):
            zs = {}
            st = sbB.tile([fb, nb, CHUNKS, 2], F32, tag="st", bufs=4,
                          name="st")
            for cc in range(CHUNKS):
                o2, xr = fpend[cc]
                relu4 = sbB.tile([128, bk], F32, tag=f"relu{cc}", bufs=2,
                                 name="relu4")
                nc.gpsimd.tensor_scalar_max(relu4, o2[:, j, :], 0.0)
                # one accumulation group for the whole bank: each transpose
                # overwrites its quarter (pending-zero from the single
                # start), the natural-layout residual matmuls accumulate.
                # Alternate between two PSUM banks so timestep j+1's
                # transposes fill while j's bank drains.
                psZ = ps.tile([fb, nb, 128], F32, tag=f"p{j % 2}_{cc}",
                              name="psZ")
                for bi in range(nb):
                    bs = slice(bi * fb, (bi + 1) * fb)
                    nc.tensor.matmul(psZ[:, bi, :], relu4[:, bs], ident_sb,
                                     is_transpose=True, start=(bi == 0),
                                     stop=False, skip_group_check=True)
                    nc.tensor.matmul(psZ[:, bi, :], xr[0:9, j, bs], wres_sb,
                                     start=False, stop=(bi == nb - 1),
                                     skip_group_check=True)
                z = sbB.tile([fb, nb, 128], BF16, tag=f"z{cc}", bufs=3,
                             name="z")
                zs[cc] = z
                zq = sbB.tile([fb, 128], BF16, tag=f"tc{cc}", name="zq")
                for bi in range(nb):
                    # natural-layout copy + LN sums on DVE; sumsq split 3:1
                    # between ScalarE (Square+accum) and DVE (ttr)
                    nc.vector.tensor_scalar(
                        z[:, bi, :], psZ[:, bi, :], 1.0, 0.0, op0=OP.mult,
                        op1=OP.add, accum_out=st[:, bi, cc, 0:1])
                    if bi < 3:
                        nc.scalar.activation(
                            out=zq, in_=psZ[:, bi, :], func=AF.Square,
                            accum_out=st[:, bi, cc, 1:2])
                    else:
                        # tensor_tensor_reduce is broken on this runtime;
                        # square via DVE mul (bf16 2x) + tensor_reduce
                        nc.vector.tensor_mul(zq, z[:, bi, :], z[:, bi, :])
                        nc.vector.tensor_reduce(
                            st[:, bi, cc, 1:2], zq,
                            axis=mybir.AxisListType.X, op=OP.add)
            # per-timestep LN stats for both chunks in one short chain —
            # no cross-t barrier, so normalize/store pipeline across t
            mu = sbB.tile([fb, nb, CHUNKS, 1], F32, tag="mu", bufs=4,
                          name="mu")
            nc.vector.tensor_scalar_mul(mu, st[:, :, :, 0:1], 1.0 / D2)
            mu2 = sbB.tile([fb, nb, CHUNKS, 1], F32, tag="mu2", bufs=4,
                           name="mu2")
            nc.vector.tensor_mul(mu2, mu, mu)
            var = sbB.tile([fb, nb, CHUNKS, 1], F32, tag="var", bufs=4,
                           name="var")
            nc.vector.scalar_tensor_tensor(var, st[:, :, :, 1:2], 1.0 / D2,
                                           mu2, op0=OP.mult, op1=OP.subtract)
            sd = sbB.tile([fb, nb, CHUNKS, 1], F32, tag="sd", bufs=4,
                          name="sd")
            nc.scalar.activation(out=sd, in_=var, func=AF.Sqrt,
                                 bias=eps_sb[0:fb, 0:1])
            rstd = sbB.tile([fb, nb, CHUNKS, 1], F32, tag="rstd", bufs=4,
                            name="rstd")
            nc.vector.reciprocal(rstd, sd)
            jj = j % Sh
            for cc in range(CHUNKS):
                ob = obufs[cc][j // Sh]
                for bi in range(nb):
                    # out = (z - mu) * rstd
                    nc.gpsimd.tensor_scalar(ob[:, bi, jj, :], zs[cc][:, bi, :],
                                            mu[:, bi, cc, 0:1],
                                            rstd[:, bi, cc, 0:1],
                                            op0=OP.subtract, op1=OP.mult)
                    if apply_gb:
                        nc.vector.tensor_mul(ob[:, bi, jj, :],
                                             ob[:, bi, jj, :], gamma_sb)
                        nc.vector.tensor_add(ob[:, bi, jj, :],
                                             ob[:, bi, jj, :], beta_sb)
            if j % Sh == Sh - 1:
                th0 = t0 + (j // Sh) * Sh
                for cc in range(CHUNKS):
                    ob = obufs[cc][j // Sh]
                    for bi in range(nb):
                        b0 = cc * bk + bi * fb
                        nc.sync.dma_start(
                            out=out_d[b0:b0 + fb, th0:th0 + Sh, :],
                            in_=ob[:, bi, :, :])
        if fnxt is not None:
            fpend = fnxt


def build(apply_gb=False, bc=BC, t_len=T, num_devices=NCORES):
    nc = bacc.Bacc("TRN2", target_bir_lowering=False, debug=False,
                   num_devices=num_devices)
    fb = min(128, bc // CHUNKS)
    D = {}

    def inp(name, shape, dtype=F32):
        D[name] = nc.dram_tensor(name, shape, dtype, kind="ExternalInput").ap()

    inp("xs", [17, t_len, bc], BF16)
    inp("rw", [128, NL, 4, 128], BF16)
    inp("pw", [128, NL - 1, 4, 2, 64], BF16)
    inp("l0w", [17, 4, 128], BF16)
    inp("br", [128, NL - 1, 3])
    inp("gb", [128, NL - 1])
    inp("wres", [9, 128], BF16)
    inp("ident", [128, 128])
    if apply_gb:
        inp("gammab", [fb, 128])
        inp("betab", [fb, 128])
    for i in range(NL):
        D[f"o{i}"] = nc.dram_tensor(f"o{i}", [128, t_len, bc], BF16).ap()
    D["out"] = nc.dram_tensor("out", [bc, t_len, 128], BF16,
                              kind="ExternalOutput").ap()

    with tile.TileContext(nc) as tc:
        with ExitStack() as ctx:
            _emit(nc, tc, ctx, D, apply_gb, bc, t_len)
    nc.compile()
    return nc


_BUILD_CACHE = {}


def kernel(x, w_ih, w_hh, b_ih, b_hh, w_res, b_res, ln_gamma, ln_beta):
    ln_gamma = np.asarray(ln_gamma, np.float32)
    ln_beta = np.asarray(ln_beta, np.float32)
    apply_gb = not (np.all(ln_gamma == 1.0) and np.all(ln_beta == 0.0))

    shared, xs_cores = _host_prep(x, w_ih, w_hh, b_ih, b_hh, w_res, b_res,
                                  NCORES, BC)
    if apply_gb not in _BUILD_CACHE:
        _BUILD_CACHE[apply_gb] = build(apply_gb)
    nc = _BUILD_CACHE[apply_gb]

    in_maps = []
    for c in range(NCORES):
        m = dict(shared)
        m["xs"] = xs_cores[c]
        if apply_gb:
            fb = min(128, BC // CHUNKS)
            m["gammab"] = np.ascontiguousarray(
                np.broadcast_to(ln_gamma, (fb, 128)).astype(np.float32))
            m["betab"] = np.ascontiguousarray(
                np.broadcast_to(ln_beta, (fb, 128)).astype(np.float32))
        in_maps.append(m)

    res = run_bass_kernel_spmd(nc, in_maps, core_ids=list(range(NCORES)))
    out = np.concatenate([res.results[c]["out"] for c in range(NCORES)], axis=0)
    return np.ascontiguousarray(out.astype(np.float32))


# revision 6
# speedup vs baseline: 2.1978x; 1.0434x over previous
"""Trainium2 Bass kernel for a 3-layer BiLSTM + ReLU + residual + LayerNorm.

Strategy (pure data parallel over 8 cores, 1024 batch rows per core):
  * Transposed on-chip layout: features on SBUF partitions, batch on the
    free dim; both directions fused on partitions (fwd 0:64, bwd 64:128).
  * All matmul operands are bf16 (1 PE cycle/row vs 4 for fp32); PSUM and
    the cell state c stay fp32.  h is produced directly in bf16 by the DVE
    (2x mode) and doubles as the DRAM layer output.
  * Layer outputs stream through DRAM in PRODUCTION order (bwd half stored
    at its production step, not its time index); the next layer's strip
    loads undo the reversal with negative-stride DMA reads.
  * All DMA is strip-granular (S=8 timesteps per transfer) to amortize the
    per-issue DGE cost; batch is processed as two interleaved 512-column
    chunks so engines overlap across the sequential per-step chain.
  * Layer 0 folds fwd+bwd input projections and both biases into a single
    K=17 matmul per gate, so its i/f/o sigmoid is a single ScalarE op.
  * Final stage: residual matmul in transposed layout, fused
    relu+add (scalar_tensor_tensor), bf16 PE transpose to natural layout,
    LN stats via DVE accumulate/reduce ops, normalize on Pool.
"""

from contextlib import ExitStack

import numpy as np
import ml_dtypes

import concourse.bacc as bacc
import concourse.tile as tile
from concourse import mybir
from concourse.bass_utils import run_bass_kernel_spmd

F32 = mybir.dt.float32
BF16 = mybir.dt.bfloat16
AF = mybir.ActivationFunctionType
OP = mybir.AluOpType

NCORES = 8
BC = 1024               # batch rows per core
CHUNKS = 2
T = 64
H = 64
NL = 3
D2 = 2 * H              # 128
S = 8                   # timesteps per DMA strip
LN_EPS = 1e-5

# gate order in PyTorch weights: i, f, g, o  (rows g*H:(g+1)*H of w_ih/w_hh)
SIG_GATES = (0, 1, 3)   # i, f, o  -> sigmoid, held in one PSUM span
TANH_GATE = 2           # g        -> tanh


def _host_prep(x, w_ih, w_hh, b_ih, b_hh, w_res, b_res, ncores, bc):
    """Matmul-ready bf16 weight layouts (shared) + per-core inputs."""
    x = np.asarray(x, np.float32)
    w_ih = np.asarray(w_ih, np.float32)
    w_hh = np.asarray(w_hh, np.float32)
    bias = np.asarray(b_ih, np.float32) + np.asarray(b_hh, np.float32)  # (NL,2,4H)
    w_res = np.asarray(w_res, np.float32)
    b_res = np.asarray(b_res, np.float32)
    t_len = x.shape[1]
    bf = ml_dtypes.bfloat16

    # Recurrent lhsT, K-major, block-diagonal over directions
    rw = np.zeros((128, NL, 4, 128), np.float32)
    for l in range(NL):
        for g in range(4):
            gs = slice(g * H, (g + 1) * H)
            rw[0:64, l, g, 0:64] = w_hh[l, 0, gs, :].T
            rw[64:128, l, g, 64:128] = w_hh[l, 1, gs, :].T

    # Input-projection lhsT for layers 1,2: pw[k, l-1, g, d, m]
    pw = np.zeros((128, NL - 1, 4, 2, 64), np.float32)
    for l in (1, 2):
        for g in range(4):
            gs = slice(g * H, (g + 1) * H)
            for d in range(2):
                pw[:, l - 1, g, d, :] = w_ih[l, d, gs, :].T

    # Layer-0 stacked projection lhsT: rows 0:8 fwd x feats -> cols 0:64,
    # row 8 ones -> both biases, rows 9:17 bwd x feats -> cols 64:128
    l0w = np.zeros((17, 4, 128), np.float32)
    for g in range(4):
        gs = slice(g * H, (g + 1) * H)
        l0w[0:8, g, 0:64] = w_ih[0, 0, gs, 0:8].T
        l0w[8, g, 0:64] = bias[0, 0, gs]
        l0w[8, g, 64:128] = bias[0, 1, gs]
        l0w[9:17, g, 64:128] = w_ih[0, 1, gs, 0:8].T

    # per-partition sigmoid-gate act biases, layers 1,2: br[p, l-1, j]
    br = np.zeros((128, NL - 1, 3), np.float32)
    for l in (1, 2):
        for j, g in enumerate(SIG_GATES):
            gs = slice(g * H, (g + 1) * H)
            br[0:64, l - 1, j] = bias[l, 0, gs]
            br[64:128, l - 1, j] = bias[l, 1, gs]

    # g-gate act bias per layer 1,2: gb[p, l-1]
    gb = np.zeros((128, NL - 1), np.float32)
    gs = slice(TANH_GATE * H, (TANH_GATE + 1) * H)
    for l in (1, 2):
        gb[0:64, l - 1] = bias[l, 0, gs]
        gb[64:128, l - 1] = bias[l, 1, gs]

    # residual lhsT (transposed layout): rows 0:8 = w_res.T, row 8 = b_res
    wres = np.zeros((9, 128), np.float32)
    wres[0:8, :] = w_res.T
    wres[8, :] = b_res

    ident = np.eye(128, dtype=np.float32)

    # Per-core transposed-augmented input xs[k, t, b] (bf16):
    # rows 0:8 x(t) feats, row 8 ones, rows 9:17 x(T-1-t) feats
    xs_cores = []
    for c in range(ncores):
        xc = x[c * bc:(c + 1) * bc]              # (bc, T, 8)
        xa = np.empty((17, t_len, bc), np.float32)
        xa[0:8] = xc.transpose(2, 1, 0)
        xa[8] = 1.0
        xa[9:17] = xc[:, ::-1].transpose(2, 1, 0)
        xs_cores.append(xa.astype(bf))

    shared = dict(rw=rw.astype(bf), pw=pw.astype(bf), l0w=l0w.astype(bf),
                  br=br, gb=gb, wres=wres.astype(bf), ident=ident)
    return shared, xs_cores


def _mm_group(nc, calls):
    """Emit a PSUM accumulation group with per-call start/stop coverage."""
    for i, (o, lh, rh, tp, rng) in enumerate(calls):
        covered = set()
        for _, _, _, _, r in calls[:i]:
            covered.update(range(*r))
        start = not set(range(*rng)).issubset(covered)
        stop = not any(max(rng[0], r[0]) < min(rng[1], r[1])
                       for _, _, _, _, r in calls[i + 1:])
        # skip_group_check: the executing-sim group checker mis-addresses
        # partition-based PSUM offsets; data semantics are still exact.
        nc.tensor.matmul(o, lh, rh, start=start, stop=stop,
                         tile_position=tp, skip_group_check=True)


def _emit(nc, tc, ctx, D, apply_gb, bc, t_len):
    bk = bc // CHUNKS
    fb = min(128, bk)         # final-stage block width (natural-layout rows)
    nb = bk // fb             # blocks per chunk per timestep
    nstrip = t_len // S

    sbC = ctx.enter_context(tc.tile_pool(name="consts", bufs=1))
    sbI = ctx.enter_context(tc.tile_pool(name="inp", bufs=2))
    sbH = ctx.enter_context(tc.tile_pool(name="hbuf", bufs=2))
    sbB = ctx.enter_context(tc.tile_pool(name="work", bufs=2))
    sbS = ctx.enter_context(tc.tile_pool(name="state", bufs=1))
    sbZ = ctx.enter_context(tc.tile_pool(name="zhold", bufs=S + 1))
    ps = ctx.enter_context(tc.tile_pool(name="ps", bufs=1, space="PSUM"))

    def const_tile(shape, dtype, key):
        t = sbC.tile(shape, dtype, name=f"c_{key}", tag=f"c_{key}")
        nc.sync.dma_start(out=t, in_=D[key])
        return t

    rw_sb = const_tile([128, NL, 4, 128], BF16, "rw")
    pw_sb = const_tile([128, NL - 1, 4, 2, 64], BF16, "pw")
    l0w_sb = const_tile([17, 4, 128], BF16, "l0w")
    br_sb = const_tile([128, NL - 1, 3], F32, "br")
    gb_sb = const_tile([128, NL - 1], F32, "gb")
    wres_sb = const_tile([9, 128], BF16, "wres")
    ident_sb = const_tile([128, 128], F32, "ident")
    gamma_sb = beta_sb = None
    if apply_gb:
        gamma_sb = const_tile([fb, 128], F32, "gammab")
        beta_sb = const_tile([fb, 128], F32, "betab")
    eps_sb = sbC.tile([128, 1], F32, name="eps")
    nc.vector.memset(eps_sb, LN_EPS)

    P = [D[f"o{i}"] for i in range(NL)]
    xs = D["xs"]
    out_d = D["out"]

    h_prev = [None] * CHUNKS   # AP of last step's h (slice of a strip buffer)
    c_st = [None] * CHUNKS

    def rev(t_hi):
        """slice reading DRAM time t_hi, t_hi-1, ..., t_hi-S+1."""
        lo = t_hi - S
        return slice(t_hi, None if lo < 0 else lo, -1)

    def issue_loads_early(l, cc, si):
        """Allocate strip tiles and issue the load halves that only depend
        on EARLY productions of layer l-1 (so they can be emitted while
        layer l-1 is still running, spreading the transition DMA burst)."""
        k0 = si * S
        cols = slice(cc * bk, cc * bk + bk)
        if l == 0:
            xt = sbI.tile([17, S, bk], BF16, tag=f"x{cc}", name="xt")
            nc.sync.dma_start(out=xt, in_=xs[:, k0:k0 + S, cols])
            return (xt,)
        src = P[l - 1]
        F = sbI.tile([128, S, bk], BF16, tag=f"F{cc}", name="Ft")
        nc.sync.dma_start(out=F[0:64], in_=src[0:64, k0:k0 + S, cols])
        B = sbI.tile([128, S, bk], BF16, tag=f"B{cc}", name="Bt")
        nc.sync.dma_start(out=B[64:128], in_=src[64:128, k0:k0 + S, cols])
        return (F, B)

    def issue_loads_late(l, cc, si, tiles):
        """Issue the time-reversed load halves (depend on LATE productions
        of layer l-1)."""
        if l == 0:
            return
        k0 = si * S
        cols = slice(cc * bk, cc * bk + bk)
        src = P[l - 1]
        F, B = tiles
        nc.sync.dma_start(out=F[64:128],
                          in_=src[64:128, rev(t_len - 1 - k0), cols])
        nc.sync.dma_start(out=B[0:64],
                          in_=src[0:64, rev(t_len - 1 - k0), cols])

    def issue_loads(l, cc, si):
        tiles = issue_loads_early(l, cc, si)
        issue_loads_late(l, cc, si, tiles)
        return tiles

    def lstm_ph1(cc, l, k, j, tiles):
        # per-gate PSUM tiles (1 bank each): gate i's next-step matmuls only
        # wait for gate i's own sigmoid to drain, not the whole step's acts
        P4 = [ps.tile([128, bk], F32, tag=f"p{gi}_{cc}", name=f"P{gi}")
              for gi in range(4)]

        def gate_calls(out_ap, g):
            if l == 0:
                (xt,) = tiles
                calls = [(out_ap, l0w_sb[:, g, :], xt[:, j, :], None, (0, 128))]
            else:
                F, B = tiles
                calls = [
                    (out_ap[0:64, :], pw_sb[:, l - 1, g, 0, :], F[:, j, :],
                     (0, 0), (0, 64)),
                    (out_ap[64:128, :], pw_sb[:, l - 1, g, 1, :], B[:, j, :],
                     (0, 64), (64, 128)),
                ]
            if k > 0:
                calls.append((out_ap, rw_sb[:, l, g, :], h_prev[cc], None,
                              (0, 128)))
            _mm_group(nc, calls)

        # emit in i, f, g, o order: the c-update chain needs i, f, g as
        # early as possible; o is only consumed after tanh(c)
        gate_calls(P4[0], SIG_GATES[0])
        gate_calls(P4[1], SIG_GATES[1])
        gate_calls(P4[3], TANH_GATE)
        gate_calls(P4[2], SIG_GATES[2])

        S4 = [sbB.tile([128, bk], BF16, tag=f"s{gi}_{cc}", name=f"S{gi}")
              for gi in range(4)]

        def sig(jg):
            bias = 0.0 if l == 0 else br_sb[:, l - 1, jg:jg + 1]
            nc.scalar.activation(out=S4[jg], in_=P4[jg], func=AF.Sigmoid,
                                 bias=bias)
        sig(0)
        sig(1)
        nc.scalar.activation(out=S4[3], in_=P4[3], func=AF.Tanh,
                             bias=0.0 if l == 0 else gb_sb[:, l - 1:l])
        sig(2)

        # chunk 0's c-chain runs on DVE, chunk 1's on Pool, so the two
        # chunks' serial c-updates execute on different engines in parallel
        eng = nc.vector if cc == 0 else nc.gpsimd
        if k == 0:
            c = sbS.tile([128, bk], F32, tag=f"c{cc}", name="c")
            c_st[cc] = c
            eng.tensor_mul(c, S4[0], S4[3])                       # c = i*g
        else:
            c = c_st[cc]
            tmp = sbB.tile([128, bk], BF16, tag=f"tmp{cc}", name="tmp")
            eng.tensor_mul(tmp, S4[0], S4[3])                     # i*g
            eng.tensor_mul(c, S4[1], c)                           # f*c
            eng.tensor_add(c, c, tmp)
        return S4, c

    def lstm_ph2(cc, S4, c, j, hb):
        # emitted after the other chunk's phase 1 so the ScalarE FIFO never
        # head-of-line blocks on tanh(c) while a ready sigmoid sits behind it
        Tc = sbB.tile([128, bk], BF16, tag=f"tc{cc}", name="Tc")
        nc.scalar.activation(out=Tc, in_=c, func=AF.Tanh)
        hs = hb[:, j, :]
        nc.vector.tensor_mul(hs, S4[2], Tc)            # h = o*tanh(c), bf16 2x
        h_prev[cc] = hs

    def fin_loads_early(cc, si):
        t0 = si * S
        cols = slice(cc * bk, cc * bk + bk)
        o2 = sbI.tile([128, S, bk], BF16, tag=f"F{cc}", name="o2")
        nc.sync.dma_start(out=o2[0:64], in_=P[2][0:64, t0:t0 + S, cols])
        xr = sbI.tile([17, S, bk], BF16, tag=f"x{cc}", name="xr")
        nc.sync.dma_start(out=xr[0:9], in_=xs[0:9, t0:t0 + S, cols])
        return (o2, xr)

    def fin_loads_late(cc, si, tiles):
        t0 = si * S
        cols = slice(cc * bk, cc * bk + bk)
        o2, xr = tiles
        nc.sync.dma_start(out=o2[64:128],
                          in_=P[2][64:128, rev(t_len - 1 - t0), cols])

    def fin_loads(cc, si):
        tiles = fin_loads_early(cc, si)
        fin_loads_late(cc, si, tiles)
        return tiles

    # ---------------- LSTM layers ----------------
    next_early = {cc: issue_loads_early(0, cc, 0) for cc in range(CHUNKS)}
    for l in range(NL):
        pend = next_early
        for cc in range(CHUNKS):
            issue_loads_late(l, cc, 0, pend[cc])
        next_early = None
        for si in range(nstrip):
            k0 = si * S
            nxt = {cc: issue_loads(l, cc, si + 1) for cc in range(CHUNKS)} \
                if si + 1 < nstrip else None
            if si == nstrip - 1:
                # the next phase's strip-0 forward-half loads only need this
                # layer's EARLY strips — issue them now to spread the
                # layer-transition DMA burst
                if l + 1 < NL:
                    next_early = {cc: issue_loads_early(l + 1, cc, 0)
                                  for cc in range(CHUNKS)}
                else:
                    fin_early = {cc: fin_loads_early(cc, 0)
                                 for cc in range(CHUNKS)}
            hb = {}
            for cc in range(CHUNKS):
                hb[cc] = sbH.tile([128, S, bk], BF16, tag=f"hb{cc}",
                                  name="hb")
            for j in range(S):
                k = k0 + j
                ph1_res = {}
                for cc in range(CHUNKS):
                    ph1_res[cc] = lstm_ph1(cc, l, k, j, pend[cc])
                for cc in range(CHUNKS):
                    S4, c = ph1_res[cc]
                    lstm_ph2(cc, S4, c, j, hb[cc])
            for cc in range(CHUNKS):
                cols = slice(cc * bk, cc * bk + bk)
                nc.sync.dma_start(out=P[l][:, k0:k0 + S, cols], in_=hb[cc])
            if nxt is not None:
                pend = nxt

    # ---- final stage: relu + residual + LayerNorm + transpose to natural ----
    fpend = fin_early
    for cc in range(CHUNKS):
        fin_loads_late(cc, 0, fpend[cc])
    Sh = S // 2    # half-strip: A/B output buffers so stores pipeline

    for si in range(nstrip):
        t0 = si * S
        fnxt = {cc: fin_loads(cc, si + 1) for cc in range(CHUNKS)} \
            if si + 1 < nstrip else None
        obufs = {}
        for cc in range(CHUNKS):
            obufs[cc] = (
                sbZ.tile([fb, nb, Sh, 128], BF16, tag=f"obA{cc}", bufs=1,
                         name="obufA"),
                sbZ.tile([fb, nb, Sh, 128], BF16, tag=f"obB{cc}", bufs=1,
                         name="obufB"),
            )
        for j in range(S):
            zs = {}
            st = sbB.tile([fb, nb, CHUNKS, 2], F32, tag="st", bufs=4,
                          name="st")
            for cc in range(CHUNKS):
                o2, xr = fpend[cc]
                relu4 = sbB.tile([128, bk], F32, tag=f"relu{cc}", bufs=2,
                                 name="relu4")
                nc.gpsimd.tensor_scalar_max(relu4, o2[:, j, :], 0.0)
                # one accumulation group for the whole bank: each transpose
                # overwrites its quarter (pending-zero from the single
                # start), the natural-layout residual matmuls accumulate.
                # Alternate between two PSUM banks so timestep j+1's
                # transposes fill while j's bank drains.
                psZ = ps.tile([fb, nb, 128], F32, tag=f"p{j % 2}_{cc}",
                              name="psZ")
                for bi in range(nb):
                    bs = slice(bi * fb, (bi + 1) * fb)
                    nc.tensor.matmul(psZ[:, bi, :], relu4[:, bs], ident_sb,
                                     is_transpose=True, start=(bi == 0),
                                     stop=False, skip_group_check=True)
                    nc.tensor.matmul(psZ[:, bi, :], xr[0:9, j, bs], wres_sb,
                                     start=False, stop=(bi == nb - 1),
                                     skip_group_check=True)
                z = sbB.tile([fb, nb, 128], BF16, tag=f"z{cc}", bufs=3,
                             name="z")
                zs[cc] = z
                zq = sbB.tile([fb, 128], BF16, tag=f"tc{cc}", name="zq")
                for bi in range(nb):
                    # natural-layout copy + LN sums on DVE; sumsq split 3:1
                    # between ScalarE (Square+accum) and DVE (ttr)
                    nc.vector.tensor_scalar(
                        z[:, bi, :], psZ[:, bi, :], 1.0, 0.0, op0=OP.mult,
                        op1=OP.add, accum_out=st[:, bi, cc, 0:1])
                    if bi < 3:
                        nc.scalar.activation(
                            out=zq, in_=psZ[:, bi, :], func=AF.Square,
                            accum_out=st[:, bi, cc, 1:2])
                    else:
                        # tensor_tensor_reduce is broken on this runtime;
                        # square via DVE mul (bf16 2x) + tensor_reduce
                        nc.vector.tensor_mul(zq, z[:, bi, :], z[:, bi, :])
                        nc.vector.tensor_reduce(
                            st[:, bi, cc, 1:2], zq,
                            axis=mybir.AxisListType.X, op=OP.add)
            # per-timestep LN stats for both chunks in one short chain —
            # no cross-t barrier, so normalize/store pipeline across t
            mu = sbB.tile([fb, nb, CHUNKS, 1], F32, tag="mu", bufs=4,
                          name="mu")
            nc.vector.tensor_scalar_mul(mu, st[:, :, :, 0:1], 1.0 / D2)
            mu2 = sbB.tile([fb, nb, CHUNKS, 1], F32, tag="mu2", bufs=4,
                           name="mu2")
            nc.vector.tensor_mul(mu2, mu, mu)
            var = sbB.tile([fb, nb, CHUNKS, 1], F32, tag="var", bufs=4,
                           name="var")
            nc.vector.scalar_tensor_tensor(var, st[:, :, :, 1:2], 1.0 / D2,
                                           mu2, op0=OP.mult, op1=OP.subtract)
            sd = sbB.tile([fb, nb, CHUNKS, 1], F32, tag="sd", bufs=4,
                          name="sd")
            nc.scalar.activation(out=sd, in_=var, func=AF.Sqrt,
                                 bias=eps_sb[0:fb, 0:1])
            rstd = sbB.tile([fb, nb, CHUNKS, 1], F32, tag="rstd", bufs=4,
                            name="rstd")
            nc.vector.reciprocal(rstd, sd)
            jj = j % Sh
            for cc in range(CHUNKS):
                ob = obufs[cc][j // Sh]
                for bi in range(nb):
                    # out = (z - mu) * rstd
                    nc.gpsimd.tensor_scalar(ob[:, bi, jj, :], zs[cc][:, bi, :],
                                            mu[:, bi, cc, 0:1],
                                            rstd[:, bi, cc, 0:1],
                                            op0=OP.subtract, op1=OP.mult)
                    if apply_gb:
                        nc.vector.tensor_mul(ob[:, bi, jj, :],
                                             ob[:, bi, jj, :], gamma_sb)
                        nc.vector.tensor_add(ob[:, bi, jj, :],
                                             ob[:, bi, jj, :], beta_sb)
            if j % Sh == Sh - 1:
                th0 = t0 + (j // Sh) * Sh
                for cc in range(CHUNKS):
                    ob = obufs[cc][j // Sh]
                    for bi in range(nb):
                        b0 = cc * bk + bi * fb
                        nc.sync.dma_start(
                            out=out_d[b0:b0 + fb, th0:th0 + Sh, :],
                            in_=ob[:, bi, :, :])
        if fnxt is not None:
            fpend = fnxt


def build(apply_gb=False, bc=BC, t_len=T, num_devices=NCORES):
    nc = bacc.Bacc("TRN2", target_bir_lowering=False, debug=False,
                   num_devices=num_devices)
    fb = min(128, bc // CHUNKS)
    D = {}

    def inp(name, shape, dtype=F32):
        D[name] = nc.dram_tensor(name, shape, dtype, kind="ExternalInput").ap()

    inp("xs", [17, t_len, bc], BF16)
    inp("rw", [128, NL, 4, 128], BF16)
    inp("pw", [128, NL - 1, 4, 2, 64], BF16)
    inp("l0w", [17, 4, 128], BF16)
    inp("br", [128, NL - 1, 3])
    inp("gb", [128, NL - 1])
    inp("wres", [9, 128], BF16)
    inp("ident", [128, 128])
    if apply_gb:
        inp("gammab", [fb, 128])
        inp("betab", [fb, 128])
    for i in range(NL):
        D[f"o{i}"] = nc.dram_tensor(f"o{i}", [128, t_len, bc], BF16).ap()
    D["out"] = nc.dram_tensor("out", [bc, t_len, 128], BF16,
                              kind="ExternalOutput").ap()

    with tile.TileContext(nc) as tc:
        with ExitStack() as ctx:
            _emit(nc, tc, ctx, D, apply_gb, bc, t_len)
    nc.compile()
    return nc


_BUILD_CACHE = {}


def kernel(x, w_ih, w_hh, b_ih, b_hh, w_res, b_res, ln_gamma, ln_beta):
    ln_gamma = np.asarray(ln_gamma, np.float32)
    ln_beta = np.asarray(ln_beta, np.float32)
    apply_gb = not (np.all(ln_gamma == 1.0) and np.all(ln_beta == 0.0))

    shared, xs_cores = _host_prep(x, w_ih, w_hh, b_ih, b_hh, w_res, b_res,
                                  NCORES, BC)
    if apply_gb not in _BUILD_CACHE:
        _BUILD_CACHE[apply_gb] = build(apply_gb)
    nc = _BUILD_CACHE[apply_gb]

    in_maps = []
    for c in range(NCORES):
        m = dict(shared)
        m["xs"] = xs_cores[c]
        if apply_gb:
            fb = min(128, BC // CHUNKS)
            m["gammab"] = np.ascontiguousarray(
                np.broadcast_to(ln_gamma, (fb, 128)).astype(np.float32))
            m["betab"] = np.ascontiguousarray(
                np.broadcast_to(ln_beta, (fb, 128)).astype(np.float32))
        in_maps.append(m)

    res = run_bass_kernel_spmd(nc, in_maps, core_ids=list(range(NCORES)))
    out = np.concatenate([res.results[c]["out"] for c in range(NCORES)], axis=0)
    return np.ascontiguousarray(out.astype(np.float32))
